# revision 1
# baseline (speedup 1.0000x reference)
"""Data-parallel 3x3 conv (NHWC 16x112x112x64, OHWI 64x3x3x64, pad=1, stride=1)
on 8 TRN2 NeuronCores via Bass/Tile.

Strategy (per core, 2 images):
  - Padded row geometry: each image row stored as 128 columns (cols 0..111 =
    data, col 112 & 127 zeroed, 113..126 don't-care), 114 rows per image
    (row 0 / 113 are zero pad). Flat position q = 128*rownum + col.
  - Input f32 NHWC is cast-DMA'd (SWDGE) to bf16 into T1 with layout
    T1[64*(rownum%2) + t, g, pos2*64 + c] = x(rownum=2g+rb, col=2t+pos2, c).
  - One batched xbar DMA-transpose per band: T2[(pos2,c), j] with pair index
    j = 64*rownum + t, i.e. partitions 0:64 = even columns (E), 64:128 = odd
    (O), free dim = pair index, linear.
  - Conv as 9 matmuls per 512-pair chunk accumulating in one PSUM bank:
    even/odd output halves live in partitions 0:64 / 64:128 of the bank.
      For dy in 0..2, D = 64*(dy-1):
        mid  both += [[W(dy,1)|W(dy,0)],[W(dy,2)|W(dy,1)]]^T @ T2[:, j0+D]  (K=128, M=128)
        so   odd  += W(dy,2)^T @ T2[:64, j0+D+1]   (K=64, E rows)
        se   even += W(dy,0)^T @ T2[64:, j0+D-1]   (K=64, O rows)
    so/se are adjacent on disjoint row halves and execute concurrently on
    the PE (the second costs ~3ns).
  - ScalarE evacuates PSUM f32 -> T3 bf16 [(pos2,co), j].
  - Second xbar transpose T3 -> T4 [64*rb+t, g, pos2*64+co].
  - SWDGE cast-DMA T4 bf16 -> f32 NHWC output.

Weights are host-packed (replicated tiny constant) into the lhsT tiles.
"""
import sys

sys.path.insert(0, "/opt/trn_rl_repo")

import ml_dtypes
import numpy as np

import concourse.bass as bass
import concourse.tile as tile
from concourse import bacc, mybir
from concourse.bass_utils import run_bass_kernel_spmd

# Problem geometry (hardcoded per spec)
N, H, W, C = 16, 112, 112, 64
NCORES = 8
NPER = N // NCORES          # images per core
ROWS = H + 2                # 114 padded rows per image
WIDTH = 128                 # padded row width (cols 0..111 data)
G_IMG = ROWS // 2           # 57 row-pairs per image
G_TOT = NPER * G_IMG        # 114
PAIRS_ROW = WIDTH // 2      # 64
PAIRS_IMG = G_IMG * WIDTH   # 7296 pairs per image
PAIRS_TOT = NPER * PAIRS_IMG  # 14592
T2_SLACK = 128              # slack pairs at each end of T2
CHUNK = 512                 # pairs per psum chunk
CHUNKS_IMG = (H * PAIRS_ROW) // CHUNK  # 14
IN_BANDS = [(0, 5), (5, 18), (18, 36), (36, 57)]
IN_BANDS_1 = [(0, 30), (30, 57)]   # img1 loads during img0 compute; fewer DMAs
OUT_BANDS = [(0, 50), (50, 57)]
OUT_BANDS_1 = [(0, 28), (28, 50), (50, 57)]

f16 = mybir.dt.bfloat16  # 16-bit compute dtype (bf16: full-rate M=128 matmul)
f32 = mybir.dt.float32


def _conv_kernel(tc, x_ap, w_ap, z_ap, y_ap):
    nc = tc.nc
    with tc.tile_pool(name="wp", bufs=1) as wp, \
         tc.tile_pool(name="big", bufs=1) as big, \
         tc.tile_pool(name="t4", bufs=2) as t4p, \
         tc.tile_pool(name="ps", bufs=8, space="PSUM") as psp:

        wt = wp.tile([128, 576], f16)   # [3 dy x 128 mid cols] + [3 dy x 64 single cols]
        nc.scalar.dma_start(wt[:], w_ap)

        T1 = big.tile([128, G_TOT, WIDTH], f16)
        T2 = big.tile([128, PAIRS_TOT + 2 * T2_SLACK], f16)
        T3 = big.tile([128, PAIRS_TOT], f16)

        T2_3d = T2[:].rearrange("p (a b) -> p a b", b=128)

        # zero T2 slack regions (read via tap offsets at image borders)
        nc.vector.memset(T2[:, 0:T2_SLACK], 0)
        nc.vector.memset(T2[:, T2_SLACK + PAIRS_TOT:], 0)

        # --- zero padding regions of T1
        pitch = T1[:].ap[0][0]
        t1t = T1[:].tensor
        # top pad rows (rownum 0 of each image): partitions 0:64, g in {0, 57}
        nc.vector.memset(
            bass.AP(t1t, 0, [[pitch, 64], [G_IMG * WIDTH, NPER], [1, WIDTH]]), 0)
        # bottom pad rows (rownum 113): partitions 64:128, g in {56, 113}
        nc.vector.memset(
            bass.AP(t1t, 64 * pitch + (G_IMG - 1) * WIDTH,
                    [[pitch, 64], [G_IMG * WIDTH, NPER], [1, WIDTH]]), 0)
        # pad cols via tiny DMAs from the zeros DRAM tensor; the first band's
        # g-range goes first so the first xbar isn't gated by the full-depth
        # pad writes
        gsplit = IN_BANDS[0][1] + 1
        for g_lo, g_n in ((0, gsplit), (gsplit, G_TOT - gsplit)):
            for p, fo in ((56, 0), (120, 0), (63, 64), (127, 64)):
                nc.scalar.dma_start(
                    bass.AP(t1t, p * pitch + fo + g_lo * WIDTH,
                            [[pitch, 1], [WIDTH, g_n], [1, 64]]),
                    z_ap[0:g_n, :])

        xt = x_ap.tensor
        yt = y_ap.tensor
        # DRAM strides (elements) for x/y [NPER, H, W, C]
        s_img, s_row = H * W * C, W * C

        def io_aps(dram_t, sbuf_tile, img, gl0, gl1, rb):
            # valid data rows y = 2*gl + rb - 1 in [0, H)
            glo = max(gl0, 1 - rb)          # rb=0 -> gl>=1 ; rb=1 -> gl>=0
            ghi = min(gl1, G_IMG - rb)      # rb=0 -> gl<=56 ; rb=1 -> gl<=55
            if glo >= ghi:
                return None, None
            ng = ghi - glo
            y0 = 2 * glo + rb - 1
            dram = bass.AP(dram_t, img * s_img + y0 * s_row,
                           [[2 * C, 56], [2 * s_row, ng], [1, WIDTH]])
            sb = sbuf_tile[64 * rb: 64 * rb + 56, img * G_IMG + glo: img * G_IMG + ghi, :]
            return dram, sb

        # ---- input: SWDGE cast DMA f32->bf16 into T1, then xbar transpose.
        # T1's pad columns/rows hold garbage; pad positions are re-zeroed in
        # T2-space per band after the transpose.
        for img in range(NPER):
            for (gl0, gl1) in (IN_BANDS if img == 0 else IN_BANDS_1):
                for rb in (0, 1):
                    dram, sb = io_aps(xt, T1, img, gl0, gl1, rb)
                    if dram is not None:
                        nc.gpsimd.dma_start(sb, dram)
                g0 = img * G_IMG + gl0
                g1 = img * G_IMG + gl1
                tin = T1[:, g0:g1, :].rearrange("p a b -> p (a b)")
                nc.sync.dma_start(T2_3d[:, 1 + g0: 1 + g1, :], tin,
                                  transpose=True)

        # ---- compute (9 matmuls + evac per chunk), with output bands
        # emitted as soon as their chunks are done
        T2v = T2[:]

        def emit_chunk(j0):
            ps = psp.tile([128, CHUNK], f32)
            for dy in range(3):
                D = 64 * (dy - 1)
                base = T2_SLACK + j0 + D
                m = 128 * dy
                sgl = 384 + 64 * dy
                # merged mid K=128 M=128
                nc.tensor.matmul(ps[:, :], wt[:, m: m + 128],
                                 T2v[:, base: base + CHUNK],
                                 start=(dy == 0), stop=False,
                                 skip_group_check=True)
                # single odd (rows E @ D+1): W(dy,2)
                nc.tensor.matmul(ps[64:128, :], wt[0:64, sgl: sgl + 64],
                                 T2v[0:64, base + 1: base + 1 + CHUNK],
                                 start=False, stop=(dy == 2), skip_group_check=True)
                # single even (rows O @ D-1): W(dy,0)
                nc.tensor.matmul(ps[0:64, :], wt[64:128, sgl: sgl + 64],
                                 T2v[64:128, base - 1: base - 1 + CHUNK],
                                 start=False, stop=(dy == 2), skip_group_check=True)
            nc.scalar.copy(T3[:, j0: j0 + CHUNK], ps[:])

        def emit_out_band(img, gl0, gl1):
            g0 = img * G_IMG + gl0
            g1 = img * G_IMG + gl1
            T4 = t4p.tile([128, gl1 - gl0, WIDTH], f16)
            nc.sync.dma_start(T4[:], T3[:, g0 * WIDTH: g1 * WIDTH], transpose=True)
            for rb in (0, 1):
                glo = max(gl0, 1 - rb)
                ghi = min(gl1, G_IMG - rb)
                if glo >= ghi:
                    continue
                ng = ghi - glo
                y0 = 2 * glo + rb - 1
                dram = bass.AP(yt, img * s_img + y0 * s_row,
                               [[2 * C, 56], [2 * s_row, ng], [1, WIDTH]])
                sb = T4[64 * rb: 64 * rb + 56, glo - gl0: ghi - gl0, :]
                nc.gpsimd.dma_start(dram, sb)

        # chunks needed before out band ending at gl1 can transpose
        def k_hi_of(gl1):
            return min(CHUNKS_IMG, (gl1 * WIDTH - PAIRS_ROW + CHUNK - 1) // CHUNK)

        for img in range(NPER):
            obands = OUT_BANDS if img == 0 else OUT_BANDS_1
            bi = 0
            for k in range(CHUNKS_IMG):
                emit_chunk(img * PAIRS_IMG + PAIRS_ROW + k * CHUNK)
                while bi < len(obands) and k + 1 >= k_hi_of(obands[bi][1]):
                    emit_out_band(img, *obands[bi])
                    bi += 1
            while bi < len(obands):
                emit_out_band(img, *obands[bi])
                bi += 1


_CACHE = {}


def _build():
    if "nc" in _CACHE:
        return _CACHE["nc"]
    nc = bacc.Bacc("TRN2", target_bir_lowering=False, debug=False,
                   num_devices=NCORES)
    x_d = nc.dram_tensor("x", [NPER * H * W * C], f32, kind="ExternalInput").ap()
    w_d = nc.dram_tensor("w", [128, 576], f16, kind="ExternalInput").ap()
    z_d = nc.dram_tensor("z", [G_TOT, 64], f16, kind="ExternalInput").ap()
    y_d = nc.dram_tensor("y", [NPER * H * W * C], f32, kind="ExternalOutput").ap()
    with tile.TileContext(nc) as tc:
        _conv_kernel(tc, x_d, w_d, z_d, y_d)
    nc.compile()
    _CACHE["nc"] = nc
    return nc


def _pack_weights(kernels):
    # kernels: (C_OUT=64, 3, 3, C_IN=64) f32, OHWI.
    # Wt[dy][dx] = [ci, co] matrix
    wt = kernels.transpose(3, 1, 2, 0).astype(ml_dtypes.bfloat16)  # [ci, dy, dx, co]
    wpk = np.zeros((128, 576), ml_dtypes.bfloat16)
    for dy in range(3):
        m = 128 * dy
        wpk[0:64, m: m + 64] = wt[:, dy, 1]        # midE even-target
        wpk[0:64, m + 64: m + 128] = wt[:, dy, 0]  # midE odd-target
        wpk[64:128, m: m + 64] = wt[:, dy, 2]      # midO even-target
        wpk[64:128, m + 64: m + 128] = wt[:, dy, 1]  # midO odd-target
        sgl = 384 + 64 * dy
        wpk[0:64, sgl: sgl + 64] = wt[:, dy, 2]    # single odd (E rows)
        wpk[64:128, sgl: sgl + 64] = wt[:, dy, 0]  # single even (O rows)
    return wpk


def kernel(x, kernels, mode=None, _trace=False, **_):
    x = np.ascontiguousarray(np.asarray(x, dtype=np.float32))
    wpk = _pack_weights(np.asarray(kernels, dtype=np.float32))
    nc = _build()
    zer = np.zeros((G_TOT, 64), ml_dtypes.bfloat16)
    in_maps = [{"x": x[i * NPER:(i + 1) * NPER].reshape(-1), "w": wpk, "z": zer}
               for i in range(NCORES)]
    res = run_bass_kernel_spmd(nc, in_maps, core_ids=list(range(NCORES)),
                               trace=_trace)
    out = np.concatenate(
        [res.results[i]["y"].reshape(NPER, H, W, C) for i in range(NCORES)], axis=0)
    if _trace:
        kernel.last_result = res
    return out.astype(np.float32)

